# revision 1
# baseline (speedup 1.0000x reference)
"""ColBERT pairwise scoring kernel for 8x TRN2 NeuronCores.

Computation (see problem reference):
    qn = l2norm(q, axis=-1); kn = l2norm(k, axis=-1)
    S[b,o,i,j] = qn[b,i,:]·kn[o,j,:], masked positions -> -inf
    s[b,o] = sum_i logsumexp_j(ALPHA*S)/ALPHA, nonfinite -> 0
    out = s / (sqrt(Lq*Lk)+1e-6) * min(exp(logit_scale), 100)

Sharding: candidate axis O is split across the 8 cores (16 o's per core);
q is replicated. Per core the kernel computes, for its o-shard:
    lse[o, bi] = sum_j exp(rk[j] * (k_raw[j,:]·qn[:,bi]))   (j = o*256..o*256+255)
where rk[j] = ALPHA/||k_j|| is folded into the ACT exp's per-partition scale
(orientation: j lives on PSUM partitions, bi on the free axis), the j-sum is
done on the tensor engine with indicator-column weights accumulating all 16
o-rows into one persistent PSUM tile, and the k-mask is handled by zeroing
masked k rows on the host (exp contributes exactly 1.0 there) and subtracting
the per-o masked count inside the final Ln's bias.

Since |ALPHA*S| <= 12, no max-subtraction is needed for a stable logsumexp.
"""

import math
import sys
from contextlib import ExitStack

import numpy as np

for _p in ("/opt/trn_rl_repo",):
    if _p not in sys.path:
        sys.path.insert(0, _p)

import concourse.bass as bass
import concourse.bacc as bacc
import concourse.tile as tile
from concourse import mybir
from concourse.bass_utils import run_bass_kernel_spmd

ALPHA = 12.0
B, Lq, O, Lk, D = 64, 32, 128, 256, 128
NCORES = 8
BI = B * Lq  # 2048 query rows, replicated on every core

F32 = mybir.dt.float32
AF = mybir.ActivationFunctionType
OP = mybir.AluOpType


def _bcast_ap(ap, parts):
    """Broadcast a [1, N] DRAM AP across `parts` partitions (step-0 AP)."""
    return bass.AP(tensor=ap.tensor, offset=ap.offset, ap=[[0, parts]] + list(ap.ap[1:]))


def emit_kernel(ctx, tc, q_d, k_d, nm_d, io_d, id_d, out_d, OL):
    """Emit the per-core program. OL = number of o's on this core."""
    nc = tc.nc
    KR = OL * Lk            # k rows on this core
    NKC = KR // 128         # k chunks (128 rows each)
    NQC = BI // 128         # q chunks = 16
    NIT = NKC               # main iterations, one per k chunk (= (o, jh))
    TG = 1024 if KR % 1024 == 0 else 512   # kt copy-group width (columns)
    NKG = KR // TG          # number of kt copy groups

    sing = ctx.enter_context(tc.tile_pool(name="sing", bufs=1))
    qnat = ctx.enter_context(tc.tile_pool(name="qnat", bufs=4))
    knat = ctx.enter_context(tc.tile_pool(name="knat", bufs=4))
    epool = ctx.enter_context(tc.tile_pool(name="epool", bufs=4))
    pm = ctx.enter_context(tc.tile_pool(name="pm", bufs=2, space="PSUM"))
    plse = ctx.enter_context(tc.tile_pool(name="plse", bufs=1, space="PSUM"))

    kt = sing.tile([128, KR], F32)      # transposed RAW k  [d, (o j)]
    qt = sing.tile([128, BI], F32)      # transposed NORMALIZED q [d, bi]
    nk = sing.tile([128, NKC], F32)     # per-k-chunk squared norms
    nq = sing.tile([128, NQC], F32)
    rk = sing.tile([128, NKC], F32)     # ALPHA / ||k row||
    rq = sing.tile([128, NQC], F32)     # 1 / ||q row||
    sqk = sing.tile([128, 128], F32)    # scratch for square outputs
    sqq = sing.tile([128, 128], F32)
    ecols = sing.tile([128, OL * OL], F32)
    id128 = sing.tile([128, 128], F32)
    negnm = sing.tile([128, 1], F32)
    loglse = sing.tile([OL, BI], F32)
    sres = sing.tile([OL, B], F32)
    bias_eps = sing.tile([128, 1], F32)   # 1e-30, keeps Ln(0) finite-input
    bias_lna = sing.tile([128, 1], F32)   # ln(ALPHA), folds alpha into rk
    nc.vector.memset(bias_eps, 1e-30)
    nc.vector.memset(bias_lna, math.log(ALPHA))

    # ---- constants in ----
    nc.sync.dma_start(out=id128, in_=id_d)
    nc.vector.memset(ecols, 0.0)
    for _o in range(OL):
        nc.vector.memset(ecols[:, _o * OL + _o:_o * OL + _o + 1], 1.0)
    nc.sync.dma_start(out=negnm, in_=nm_d)

    # ---- input loads: q quarters interleaved with k groups (512 rows each) ---
    qtiles = []
    ktiles = []
    NKLG = KR // 512
    for g in range(max(4, NKLG)):
        if g < 4:
            t = qnat.tile([128, 4, 128], F32, tag="qn")
            nc.sync.dma_start(
                out=t, in_=q_d[g * 512:(g + 1) * 512, :].rearrange("(c p) d -> p c d", p=128)
            )
            qtiles.append(t)
        if g < NKLG:
            t = knat.tile([128, 4, 128], F32, tag="kn")
            nc.sync.dma_start(
                out=t, in_=k_d[g * 512:(g + 1) * 512, :].rearrange("(c p) d -> p c d", p=128)
            )
            ktiles.append(t)

    # ---- q pipeline: norms -> rq -> scale -> transpose -> qt (per quarter) ----
    # DVE: per-chunk squared-norm accumulate; ACT: rq = exp(-0.5*ln(n2+eps)).
    for g in range(4):
        for c in range(4):
            cc = 4 * g + c
            nc.vector.tensor_mul(sqq, qtiles[g][:, c, :], qtiles[g][:, c, :])
            nc.vector.reduce_sum(out=nq[:, cc:cc + 1], in_=sqq,
                                 axis=mybir.AxisListType.X)
        nc.scalar.activation(out=rq[:, g * 4:(g + 1) * 4], in_=nq[:, g * 4:(g + 1) * 4],
                             func=AF.Sqrt, bias=bias_eps[:, 0:1], scale=1.0)
        nc.vector.reciprocal(out=rq[:, g * 4:(g + 1) * 4], in_=rq[:, g * 4:(g + 1) * 4])
        for c in range(4):
            cc = 4 * g + c
            nc.vector.tensor_scalar(
                out=qtiles[g][:, c, :], in0=qtiles[g][:, c, :],
                scalar1=rq[:, cc:cc + 1], scalar2=None, op0=OP.mult,
            )

    # ---- k norms (DVE) + rk (ACT, two batches; second emitted mid-loop) ----
    def emit_k_norms(c0, c1):
        for cc in range(c0, c1):
            nc.vector.tensor_mul(sqk, ktiles[cc // 4][:, cc % 4, :],
                                 ktiles[cc // 4][:, cc % 4, :])
            nc.vector.reduce_sum(out=nk[:, cc:cc + 1], in_=sqk,
                                 axis=mybir.AxisListType.X)

    def emit_rk(c0, c1):
        nc.scalar.activation(out=rk[:, c0:c1], in_=nk[:, c0:c1],
                             func=AF.Sqrt, bias=bias_eps[:, 0:1], scale=1.0)
        nc.vector.reciprocal(out=rk[:, c0:c1], in_=rk[:, c0:c1])
        nc.vector.tensor_scalar_mul(rk[:, c0:c1], rk[:, c0:c1], float(ALPHA))

    emit_k_norms(0, NKC // 2)
    emit_rk(0, NKC // 2)
    emit_k_norms(NKC // 2, NKC)

    # ---- transposes: PE transpose [128,128] blocks into PSUM, DVE copy out ---
    def transpose_group(dst, dst_off, src_tiles, src_chunk0, nchunks):
        """Transpose `nchunks` natural chunks into dst[:, dst_off:dst_off+128*n]."""
        for i in range(nchunks):
            cc = src_chunk0 + i
            pt = pm.tile([128, 128], F32, tag="mm")
            nc.tensor.transpose(
                out=pt, in_=src_tiles[cc // 4][:, cc % 4, :], identity=id128)
            nc.vector.tensor_copy(
                out=dst[:, dst_off + i * 128: dst_off + (i + 1) * 128], in_=pt)

    # k groups 0..1 first (needed by early mains), then q, then rest of k later
    early_kg = min(2, NKG)
    for g in range(early_kg):
        transpose_group(kt, g * TG, ktiles, g * (TG // 128), TG // 128)
    for g in range(4):
        transpose_group(qt, g * 512, qtiles, g * 4, 4)

    # ---- main loop: software-pipelined matmul -> exp -> reduce-matmul ----
    lse = plse.tile([OL, BI], F32)
    et = {}
    for it in range(NIT + 1):
        if it == 4 and NKG > early_kg:
            for g in range(early_kg, NKG):
                transpose_group(kt, g * TG, ktiles, g * (TG // 128), TG // 128)
        if it == NIT // 2:
            emit_rk(NKC // 2, NKC)
        if it < NIT:
            o = it // 2
            ts = []
            es = []
            for h in range(2):
                T = pm.tile([128, 1024], F32, tag="mm")
                for s2 in range(2):
                    nc.tensor.matmul(
                        out=T[:, s2 * 512:(s2 + 1) * 512],
                        lhsT=kt[:, it * 128:(it + 1) * 128],
                        rhs=qt[:, h * 1024 + s2 * 512: h * 1024 + (s2 + 1) * 512],
                        start=True, stop=True,
                    )
                ts.append(T)
            for h in range(2):
                e = epool.tile([128, 1024], F32, tag="e")
                nc.scalar.activation(out=e, in_=ts[h], func=AF.Exp,
                                     bias=0.0, scale=rk[:, it:it + 1])
                es.append(e)
            et[it] = es
        if it > 0:
            p = it - 1
            o_p = p // 2
            for h, e in enumerate(et.pop(p)):
                for s2 in range(2):
                    nc.tensor.matmul(
                        out=lse[0:OL, h * 1024 + s2 * 512: h * 1024 + (s2 + 1) * 512],
                        lhsT=ecols[:, o_p * OL:(o_p + 1) * OL],
                        rhs=e[:, s2 * 512:(s2 + 1) * 512],
                        start=(p == 0), stop=(p == NIT - 1),
                    )

    # ---- tail: log(sum - n_masked), sum over Lq, store ----
    nc.scalar.activation(out=loglse, in_=lse[0:OL, :], func=AF.Ln,
                         bias=negnm[0:OL, 0:1], scale=1.0)
    nc.vector.tensor_reduce(
        out=sres, in_=loglse.rearrange("p (b i) -> p b i", i=Lq),
        axis=mybir.AxisListType.X, op=OP.add,
    )
    nc.sync.dma_start(out=out_d, in_=sres)


def build_program(OL):
    KR = OL * Lk
    nc = bacc.Bacc("TRN2", target_bir_lowering=False, debug=False,
                   enable_asserts=False, num_devices=NCORES)
    q_d = nc.dram_tensor("q_in", [BI, D], F32, kind="ExternalInput").ap()
    k_d = nc.dram_tensor("k_in", [KR, D], F32, kind="ExternalInput").ap()
    nm_d = nc.dram_tensor("negnm", [128, 1], F32, kind="ExternalInput").ap()
    id_d = nc.dram_tensor("id128", [128, 128], F32, kind="ExternalInput").ap()
    out_d = nc.dram_tensor("outp", [OL, B], F32, kind="ExternalOutput").ap()

    with tile.TileContext(nc) as tc, ExitStack() as ctx:
        emit_kernel(ctx, tc, q_d, k_d, nm_d, None, id_d, out_d, OL)
    nc.compile()
    return nc


def make_in_maps(q, k, k_mask, OL, ncores):
    """Host-side shard prep. Returns per-core input dicts."""
    qf = np.ascontiguousarray(q.reshape(BI, D), dtype=np.float32)
    kz = np.ascontiguousarray(k, dtype=np.float32).copy()
    kz[k_mask.astype(bool)] = 0.0
    nmask = k_mask.astype(bool).sum(axis=1).astype(np.float32)  # [O]
    id128 = np.eye(128, dtype=np.float32)
    in_maps = []
    for c in range(ncores):
        osl = slice(c * OL, (c + 1) * OL)
        in_maps.append({
            "q_in": qf,
            "k_in": np.ascontiguousarray(kz[osl].reshape(OL * Lk, D)),
            "negnm": np.ascontiguousarray(
                np.pad(-nmask[osl], (0, 128 - OL)).reshape(128, 1)),
            "id128": id128,
        })
    return in_maps


def postprocess(per_core_out, q_mask, k_mask, logit_scale, OL, ncores):
    """Gather per-core [OL, B] results into the final [B, O] output."""
    s = np.empty((B, ncores * OL), dtype=np.float32)
    for c in range(ncores):
        s[:, c * OL:(c + 1) * OL] = per_core_out[c].T
    coef = min(math.exp(float(logit_scale)), 100.0) / (
        ALPHA * (math.sqrt(Lq * Lk) + 1e-06))
    s = s * np.float32(coef)
    # rows with any masked query token are -inf in the reference -> zeroed
    s[np.asarray(q_mask).astype(bool).any(axis=1), :] = 0.0
    # fully-masked candidates are -inf in the reference -> zeroed
    s[:, np.asarray(k_mask).astype(bool).all(axis=1)] = 0.0
    s = np.where(np.isfinite(s), s, 0.0).astype(np.float32)
    return s


_CACHED_NC = None


def kernel(q, k, q_mask, k_mask, logit_scale):
    global _CACHED_NC
    OL = O // NCORES
    if _CACHED_NC is None:
        _CACHED_NC = build_program(OL)
    in_maps = make_in_maps(np.asarray(q), np.asarray(k), np.asarray(k_mask), OL, NCORES)
    res = run_bass_kernel_spmd(_CACHED_NC, in_maps, list(range(NCORES)))
    outs = [np.asarray(res.results[c]["outp"]) for c in range(NCORES)]
    return postprocess(outs, q_mask, k_mask, logit_scale, OL, NCORES)



# revision 5
# speedup vs baseline: 7.2566x; 7.2566x over previous
"""ColBERT pairwise scoring kernel for 8x TRN2 NeuronCores.

Computation (see problem reference):
    qn = l2norm(q, axis=-1); kn = l2norm(k, axis=-1)
    S[b,o,i,j] = qn[b,i,:]·kn[o,j,:], masked positions -> -inf
    s[b,o] = sum_i logsumexp_j(ALPHA*S)/ALPHA, nonfinite -> 0
    out = s / (sqrt(Lq*Lk)+1e-6) * min(exp(logit_scale), 100)

Key observations exploited here:
  * Any batch row b with >= 1 masked query token is exactly 0 in the
    reference output (the -inf from that token survives the sum over Lq and
    is then zeroed).  Only rows with a fully-unmasked query need computing.
    The host packs those rows (up to QB_CAP=8 of them per pass) into a
    [256, D] tile; if more survive, the same program is run multiple times.
  * bf16 matmuls run 4x faster than fp32 on the PE (1 cycle/col vs 4), and
    |S| <= 1 with unit-norm rows, so bf16 inputs keep the overall relative
    error ~1e-3, far inside the 2e-2 gate.  No max-subtraction is needed for
    the logsumexp since |ALPHA*S| <= 12.

Sharding: candidate axis O split across 8 cores (OL=16 o's per core), packed
q replicated.  Per-core pipeline (all bf16 except PSUM):
    k arrives paired ([2048, 256] view of [4096, 128] so DMA descriptors are
    512B) -> row norms on GPSIMD -> rk = ALPHA/||k|| via exp(-0.5*ln) on ACT
    -> DVE scales k rows by rk -> PE transposes to kt[d, j] -> main matmul
    kt_chunk^T @ qt -> ACT exp over [128,1024] groups -> indicator-column
    matmul accumulates per-o sums into one PSUM tile -> Ln(sum - n_masked)
    -> sum over Lq -> DMA out [OL, QB_CAP].
Masked k rows are zeroed on the host (exp contributes exactly 1.0 there) and
the per-o masked count is subtracted inside the final Ln's bias.
"""

import math
import sys
from contextlib import ExitStack

import ml_dtypes
import numpy as np

BF16NP = ml_dtypes.bfloat16

for _p in ("/opt/trn_rl_repo",):
    if _p not in sys.path:
        sys.path.insert(0, _p)

import concourse.bass as bass
import concourse.bacc as bacc
import concourse.tile as tile
from concourse import mybir
from concourse.bass_utils import run_bass_kernel_spmd

ALPHA = 12.0
B, Lq, O, Lk, D = 64, 32, 128, 256, 128
NCORES = 8
OL = O // NCORES          # candidates per core = 16
KR = OL * Lk              # k rows per core = 4096
NKP = KR // 256           # paired k chunks per core = 16
QB_CAP = 8                # max surviving batch rows per pass
BIP = QB_CAP * Lq         # packed query rows = 256

F32 = mybir.dt.float32
BF16 = mybir.dt.bfloat16
AF = mybir.ActivationFunctionType
OP = mybir.AluOpType


def emit_kernel(ctx, tc, q_d, k_d, nm_d, ec_d, id_d, out_d):
    nc = tc.nc
    NCH = KR // 128           # 128-row j chunks = 32
    NG = NCH // 4             # exp groups of 4 chunks = 8

    sing = ctx.enter_context(tc.tile_pool(name="sing", bufs=1))
    kscp = ctx.enter_context(tc.tile_pool(name="ksc", bufs=4))
    pmt = ctx.enter_context(tc.tile_pool(name="pmt", bufs=2, space="PSUM"))
    pm = ctx.enter_context(tc.tile_pool(name="pm", bufs=2, space="PSUM"))
    plse = ctx.enter_context(tc.tile_pool(name="plse", bufs=1, space="PSUM"))
    etp = ctx.enter_context(tc.tile_pool(name="et", bufs=2))

    kin = sing.tile([128, NKP, 256], BF16)   # paired natural k
    qin = sing.tile([128, 2, 128], BF16)     # natural packed q
    kt = sing.tile([128, KR], BF16)          # scaled transposed k [d, j]
    qt = sing.tile([128, BIP], BF16)         # normalized transposed q [d, bi]
    nk = sing.tile([128, 2 * NKP], F32)      # k row squared norms
    rk = sing.tile([128, 2 * NKP], F32)      # ALPHA / ||k row||
    nq = sing.tile([128, 2], F32)
    rq = sing.tile([128, 2], F32)
    sqp = sing.tile([128, 128], BF16)        # gpsimd STT scratch out
    sqv = sing.tile([128, 128], BF16)        # dve STT scratch out
    ecols = sing.tile([128, OL * OL], BF16)  # indicator columns (host-built)
    id128 = sing.tile([128, 128], BF16)
    negnm = sing.tile([128, 1], F32)
    loglse = sing.tile([OL, BIP], F32)
    sres = sing.tile([OL, QB_CAP], F32)
    bias_eps = sing.tile([128, 1], F32)      # 1e-30, keeps Ln input nonzero
    bias_lna = sing.tile([128, 1], F32)      # ln(ALPHA), folds alpha into rk
    nc.vector.memset(bias_eps, 1e-30)
    nc.vector.memset(bias_lna, math.log(ALPHA))

    # ---- inputs in: small constants, q, then k in two halves ----
    nc.sync.dma_start(out=id128, in_=id_d)
    nc.sync.dma_start(out=ecols, in_=ec_d)
    nc.sync.dma_start(out=negnm, in_=nm_d)
    nc.sync.dma_start(out=qin, in_=q_d.rearrange("(c p) d -> p c d", p=128))
    kre = k_d.rearrange("(c p) e -> p c e", p=128)
    nc.sync.dma_start(out=kin[:, 0:NKP // 2, :], in_=kre[:, 0:NKP // 2, :])
    nc.sync.dma_start(out=kin[:, NKP // 2:NKP, :], in_=kre[:, NKP // 2:NKP, :])

    # ---- q pipeline: norms (DVE) -> rq = exp(-0.5 ln) (ACT) -> scale,
    #      transpose, copy into qt ----
    for c in range(2):
        nc.vector.scalar_tensor_tensor(
            out=sqv, in0=qin[:, c, :], scalar=1.0, in1=qin[:, c, :],
            op0=OP.mult, op1=OP.mult, accum_out=nq[:, c:c + 1])
    nc.scalar.activation(out=nq, in_=nq, func=AF.Ln,
                         bias=bias_eps[:, 0:1], scale=1.0)
    nc.scalar.activation(out=rq, in_=nq, func=AF.Exp, bias=0.0, scale=-0.5)
    for c in range(2):
        qs = kscp.tile([128, 128], BF16, tag="qs")
        nc.vector.tensor_scalar(out=qs, in0=qin[:, c, :],
                                scalar1=rq[:, c:c + 1], scalar2=None,
                                op0=OP.mult)
        pq = pmt.tile([128, 128], BF16, tag="pt")
        nc.tensor.transpose(out=pq, in_=qs, identity=id128)
        nc.vector.tensor_copy(out=qt[:, c * 128:(c + 1) * 128], in_=pq)

    # ---- k norms on GPSIMD (otherwise idle), in DMA-half order ----
    def emit_knorms(p0, p1):
        for c in range(p0, p1):
            for s in range(2):
                nc.vector.scalar_tensor_tensor(
                    out=sqp, in0=kin[:, c, s * 128:(s + 1) * 128], scalar=1.0,
                    in1=kin[:, c, s * 128:(s + 1) * 128],
                    op0=OP.mult, op1=OP.mult,
                    accum_out=nk[:, 2 * c + s:2 * c + s + 1])

    def emit_rk(c0, c1):
        # rk = ALPHA * exp(-0.5 * ln(n2)): stays within the ln/exp ACT table
        nc.scalar.activation(out=nk[:, 2 * c0:2 * c1], in_=nk[:, 2 * c0:2 * c1],
                             func=AF.Ln, bias=bias_eps[:, 0:1], scale=1.0)
        nc.scalar.activation(out=rk[:, 2 * c0:2 * c1], in_=nk[:, 2 * c0:2 * c1],
                             func=AF.Exp, bias=bias_lna[:, 0:1], scale=-0.5)

    def emit_kprep(c):
        # scale both sub-rows of pair c by rk, transpose into kt[:, c*256:...]
        ks = kscp.tile([128, 2, 128], BF16, tag="ks")
        pt = pmt.tile([128, 256], BF16, tag="pt")
        for s in range(2):
            nc.vector.tensor_scalar(
                out=ks[:, s, :], in0=kin[:, c, s * 128:(s + 1) * 128],
                scalar1=rk[:, 2 * c + s:2 * c + s + 1], scalar2=None,
                op0=OP.mult)
            nc.tensor.transpose(out=pt[:, s * 128:(s + 1) * 128],
                                in_=ks[:, s, :], identity=id128)
        nc.vector.tensor_copy(out=kt[:, c * 256:(c + 1) * 256], in_=pt)

    emit_knorms(0, NKP // 2)
    emit_rk(0, NKP // 2)
    emit_knorms(NKP // 2, NKP)
    for c in range(NKP // 2):
        emit_kprep(c)

    # ---- main loop: per group of 4 j-chunks: matmul -> exp -> reduce ----
    lse = plse.tile([OL, BIP], F32)
    second_half_done = False
    for g in range(NG):
        if g == 2 and not second_half_done:
            # kt pairs 8..15 are needed from group 4 on; prep them now so
            # DVE/PE work overlaps the earlier groups' ACT time.
            emit_rk(NKP // 2, NKP)
            for c in range(NKP // 2, NKP):
                emit_kprep(c)
            second_half_done = True
        T = pm.tile([128, 4 * BIP], F32, tag="mm")
        for s in range(4):
            ch = 4 * g + s
            nc.tensor.matmul(
                out=T[:, s * BIP:(s + 1) * BIP],
                lhsT=kt[:, ch * 128:(ch + 1) * 128],
                rhs=qt, start=True, stop=True)
        e = etp.tile([128, 4 * BIP], BF16, tag="e")
        nc.scalar.activation(out=e, in_=T, func=AF.Exp, bias=0.0, scale=1.0)
        for s in range(4):
            ch = 4 * g + s
            o = ch // 2
            nc.tensor.matmul(
                out=lse[0:OL, :],
                lhsT=ecols[:, o * OL:(o + 1) * OL],
                rhs=e[:, s * BIP:(s + 1) * BIP],
                start=(g == 0 and s == 0), stop=(g == NG - 1 and s == 3))

    # ---- tail: log(sum - n_masked), sum over Lq, store ----
    nc.scalar.activation(out=loglse, in_=lse[0:OL, :], func=AF.Ln,
                         bias=negnm[0:OL, 0:1], scale=1.0)
    nc.vector.tensor_reduce(
        out=sres, in_=loglse.rearrange("p (b i) -> p b i", i=Lq),
        axis=mybir.AxisListType.X, op=OP.add)
    nc.sync.dma_start(out=out_d, in_=sres)


def build_program():
    nc = bacc.Bacc("TRN2", target_bir_lowering=False, debug=False,
                   enable_asserts=False, num_devices=NCORES)
    q_d = nc.dram_tensor("q_in", [BIP, D], BF16, kind="ExternalInput").ap()
    k_d = nc.dram_tensor("k_in", [KR // 2, 2 * D], BF16, kind="ExternalInput").ap()
    nm_d = nc.dram_tensor("negnm", [128, 1], F32, kind="ExternalInput").ap()
    ec_d = nc.dram_tensor("ecols", [128, OL * OL], BF16, kind="ExternalInput").ap()
    id_d = nc.dram_tensor("id128", [128, 128], BF16, kind="ExternalInput").ap()
    out_d = nc.dram_tensor("outp", [OL, QB_CAP], F32, kind="ExternalOutput").ap()

    with tile.TileContext(nc) as tc, ExitStack() as ctx:
        emit_kernel(ctx, tc, q_d, k_d, nm_d, ec_d, id_d, out_d)
    nc.compile()
    return nc


def make_in_maps(q, k, q_mask, k_mask, OL_=None, ncores=NCORES):
    """Host-side shard prep.  Returns (passes, groups): passes is a list of
    per-core input-dict lists (one entry per device pass), groups the list of
    surviving batch indices handled by each pass."""
    q = np.asarray(q, dtype=np.float32)
    k = np.asarray(k, dtype=np.float32)
    q_mask = np.asarray(q_mask).astype(bool)
    k_mask = np.asarray(k_mask).astype(bool)

    surv = np.nonzero(~q_mask.any(axis=1))[0]
    groups = [surv[i:i + QB_CAP] for i in range(0, len(surv), QB_CAP)]
    if not groups:
        groups = [np.zeros((0,), dtype=np.int64)]

    kz = k.copy()
    kz[k_mask] = 0.0
    nmask = k_mask.sum(axis=1).astype(np.float32)           # [O]
    id128 = np.eye(128, dtype=BF16NP)
    ec = np.zeros((128, OL * OL), dtype=BF16NP)
    for o in range(OL):
        ec[:, o * OL + o] = 1.0

    core_static = []
    for c in range(ncores):
        osl = slice(c * OL, (c + 1) * OL)
        core_static.append({
            "k_in": np.ascontiguousarray(
                kz[osl].reshape(KR // 2, 2 * D).astype(BF16NP)),
            "negnm": np.ascontiguousarray(
                np.pad(-nmask[osl], (0, 128 - OL)).reshape(128, 1)),
            "ecols": ec,
            "id128": id128,
        })

    passes = []
    for g in groups:
        qp = np.zeros((BIP, D), dtype=np.float32)
        if len(g):
            qp[:len(g) * Lq] = q[g].reshape(len(g) * Lq, D)
        qp = qp.astype(BF16NP)
        in_maps = []
        for c in range(ncores):
            m = dict(core_static[c])
            m["q_in"] = qp
            in_maps.append(m)
        passes.append(in_maps)
    return passes, groups


def postprocess(per_pass_outs, groups, k_mask, logit_scale, ncores=NCORES):
    """Scatter per-pass [OL, QB_CAP] core results into the [B, O] output."""
    out = np.zeros((B, O), dtype=np.float32)
    coef = min(math.exp(float(logit_scale)), 100.0) / (
        ALPHA * (math.sqrt(Lq * Lk) + 1e-06))
    for outs, g in zip(per_pass_outs, groups):
        nb = len(g)
        if nb == 0:
            continue
        for c in range(ncores):
            blk = np.asarray(outs[c])[:, :nb].T * np.float32(coef)  # [nb, OL]
            out[np.asarray(g), c * OL:(c + 1) * OL] = blk
    out[:, np.asarray(k_mask).astype(bool).all(axis=1)] = 0.0
    return np.where(np.isfinite(out), out, 0.0).astype(np.float32)


_CACHED_NC = None


def kernel(q, k, q_mask, k_mask, logit_scale):
    global _CACHED_NC
    if _CACHED_NC is None:
        _CACHED_NC = build_program()
    passes, groups = make_in_maps(q, k, q_mask, k_mask)
    per_pass_outs = []
    for in_maps in passes:
        res = run_bass_kernel_spmd(_CACHED_NC, in_maps, list(range(NCORES)))
        per_pass_outs.append(
            [np.asarray(res.results[c]["outp"]) for c in range(NCORES)])
    return postprocess(per_pass_outs, groups, k_mask, logit_scale)


# revision 35
# speedup vs baseline: 9.6857x; 1.3348x over previous
"""ColBERT pairwise scoring kernel for 8x TRN2 NeuronCores.

Computation (see problem reference):
    qn = l2norm(q, axis=-1); kn = l2norm(k, axis=-1)
    S[b,o,i,j] = qn[b,i,:]·kn[o,j,:], masked positions -> -inf
    s[b,o] = sum_i logsumexp_j(ALPHA*S)/ALPHA, nonfinite -> 0
    out = s / (sqrt(Lq*Lk)+1e-6) * min(exp(logit_scale), 100)

Key observations exploited here:
  * Any batch row b with >= 1 masked query token is exactly 0 in the
    reference output (the -inf from that token survives the sum over Lq and
    is then zeroed).  Only rows with a fully-unmasked query need computing.
    The host packs those rows (up to QB_CAP=8 of them per pass) into a
    [256, D] tile; if more survive, the same program is run multiple times.
  * bf16 matmuls run 4x faster than fp32 on the PE (1 cycle/col vs 4), and
    |S| <= 1 with unit-norm rows, so bf16 inputs keep the overall relative
    error ~1e-3, far inside the 2e-2 gate.  No max-subtraction is needed for
    the logsumexp since |ALPHA*S| <= 12.

Sharding: candidate axis O split across 8 cores (OL=16 o's per core), packed
q replicated.  Per-core pipeline (all bf16 except PSUM):
    k arrives paired ([2048, 256] view of [4096, 128] so DMA descriptors are
    512B) -> row norms on GPSIMD -> rk = ALPHA/||k|| via exp(-0.5*ln) on ACT
    -> DVE scales k rows by rk -> PE transposes to kt[d, j] -> main matmul
    kt_chunk^T @ qt -> ACT exp over [128,1024] groups -> indicator-column
    matmul accumulates per-o sums into one PSUM tile -> Ln(sum - n_masked)
    -> sum over Lq -> DMA out [OL, QB_CAP].
Masked k rows are zeroed on the host (exp contributes exactly 1.0 there) and
the per-o masked count is subtracted inside the final Ln's bias.
"""

import math
import sys
from contextlib import ExitStack

import ml_dtypes
import numpy as np

BF16NP = ml_dtypes.bfloat16

for _p in ("/opt/trn_rl_repo",):
    if _p not in sys.path:
        sys.path.insert(0, _p)

import concourse.bass as bass
import concourse.bacc as bacc
import concourse.tile as tile
from concourse import mybir
from concourse.bass_utils import run_bass_kernel_spmd

ALPHA = 12.0
B, Lq, O, Lk, D = 64, 32, 128, 256, 128
NCORES = 8
OL = O // NCORES          # candidates per core = 16
KR = OL * Lk              # k rows per core = 4096
NKP = KR // 256           # paired k chunks per core = 16
QB_CAP = 8                # max surviving batch rows per pass
BIP = QB_CAP * Lq         # packed query rows = 256

F32 = mybir.dt.float32
BF16 = mybir.dt.bfloat16
AF = mybir.ActivationFunctionType
OP = mybir.AluOpType


def emit_kernel(ctx, tc, q_d, k_d, cd_d, out_d):
    nc = tc.nc
    NCH = KR // 128           # 128-row j chunks = 32
    NG = NCH // 4             # exp groups of 4 chunks = 8
    NQD = 4                   # k arrives in 4 quarter DMAs
    QP = NKP // NQD           # paired chunks per quarter = 4

    sing = ctx.enter_context(tc.tile_pool(name="sing", bufs=1))
    kscp = ctx.enter_context(tc.tile_pool(name="ksc", bufs=4))
    sqp = ctx.enter_context(tc.tile_pool(name="sq", bufs=6))
    pmt = ctx.enter_context(tc.tile_pool(name="pmt", bufs=2, space="PSUM"))
    pm = ctx.enter_context(tc.tile_pool(name="pm", bufs=2, space="PSUM"))
    plse = ctx.enter_context(tc.tile_pool(name="plse", bufs=2, space="PSUM"))
    etp = ctx.enter_context(tc.tile_pool(name="et", bufs=2))

    kin = sing.tile([128, NKP, 256], BF16)   # paired natural k
    qin = sing.tile([128, 2, 128], BF16)     # natural packed q
    kt = sing.tile([128, KR], BF16)          # scaled transposed k [d, j]
    qt = sing.tile([128, BIP], BF16)         # normalized transposed q [d, bi]
    nk = sing.tile([128, 2 * NKP], F32)      # k row squared norms
    rk = sing.tile([128, 2 * NKP], F32)      # ALPHA / ||k row||
    nq = sing.tile([128, 2], F32)
    rq = sing.tile([128, 2], F32)
    cdat = sing.tile([128, 128 + OL * OL + 2], BF16)  # id128 ++ ecols ++ negnm
    id128 = cdat[:, 0:128]
    ecols = cdat[:, 128:128 + OL * OL]
    negnm = sing.tile([128, 2], F32)   # col h: -n_masked for o in half h
    loglse = sing.tile([OL // 2, BIP], F32)
    sres = sing.tile([OL // 2, 2 * QB_CAP], F32)
    bias_eps = sing.tile([128, 1], F32)      # 1e-30, keeps Ln input nonzero
    bias_lna = sing.tile([128, 1], F32)      # ln(ALPHA), folds alpha into rk
    nc.vector.memset(bias_eps, 1e-30)
    nc.vector.memset(bias_lna, math.log(ALPHA))

    # ---- inputs in: q first (its prep finishes before k lands), then k
    #      quarters, all on SP; constants ride the ACT queue ----
    kre = k_d.rearrange("(c p) e -> p c e", p=128)
    nc.sync.dma_start(out=qin, in_=q_d.rearrange("(c p) d -> p c d", p=128))
    for qd in range(NQD):
        nc.sync.dma_start(out=kin[:, qd * QP:(qd + 1) * QP, :],
                          in_=kre[:, qd * QP:(qd + 1) * QP, :])
    nc.scalar.dma_start(out=cdat, in_=cd_d)

    # ---- helpers ----
    def emit_knorms(p0, p1):
        for c in range(p0, p1):
            for s in range(2):
                sq = sqp.tile([128, 128], BF16, tag="sq")
                nc.vector.scalar_tensor_tensor(
                    out=sq, in0=kin[:, c, s * 128:(s + 1) * 128], scalar=1.0,
                    in1=kin[:, c, s * 128:(s + 1) * 128],
                    op0=OP.mult, op1=OP.mult,
                    accum_out=nk[:, 2 * c + s:2 * c + s + 1])

    def emit_rk(c0, c1):
        # rk = ALPHA * exp(-0.5 * ln(n2)): stays within the ln/exp ACT table
        nc.scalar.activation(out=nk[:, 2 * c0:2 * c1], in_=nk[:, 2 * c0:2 * c1],
                             func=AF.Ln, bias=bias_eps[:, 0:1], scale=1.0)
        nc.scalar.activation(out=rk[:, 2 * c0:2 * c1], in_=nk[:, 2 * c0:2 * c1],
                             func=AF.Exp, bias=bias_lna[:, 0:1], scale=-0.5)

    # ---- q prep first (qin lands before k), then per-quad norm->rk->prep
    #      so quad 0 feeds the first main matmul as early as possible ----
    for c in range(2):
        sq = sqp.tile([128, 128], BF16, tag="sq")
        nc.vector.scalar_tensor_tensor(
            out=sq, in0=qin[:, c, :], scalar=1.0, in1=qin[:, c, :],
            op0=OP.mult, op1=OP.mult, accum_out=nq[:, c:c + 1])
    nc.scalar.activation(out=nq, in_=nq, func=AF.Ln,
                         bias=bias_eps[:, 0:1], scale=1.0)
    nc.scalar.activation(out=rq, in_=nq, func=AF.Exp, bias=0.0, scale=-0.5)
    qs = kscp.tile([128, 2, 128], BF16, tag="qs")
    pq = pmt.tile([128, 1024], BF16, tag="pt")
    for c in range(2):
        nc.vector.tensor_scalar(out=qs[:, c, :], in0=qin[:, c, :],
                                scalar1=rq[:, c:c + 1], scalar2=None,
                                op0=OP.mult)
        nc.tensor.transpose(out=pq[:, c * 128:(c + 1) * 128],
                            in_=qs[:, c, :], identity=id128)
    nc.vector.tensor_copy(out=qt, in_=pq[:, 0:256])

    # k prep: per quad qd (pairs 2qd, 2qd+1 -> kt cols qd*512..) norms, rk,
    # scale, transpose into an "oct" [128, 1024] PSUM tile shared by quad
    # pairs; one copy per oct (ACT pre-loop where it idles, DVE mid-loop).
    octs = {}

    def emit_kprep_quad(qd, copy_eng, single=False):
        # single=True: quad gets its own [128, 512] copy right away (lower
        # latency for the first mains); else pairs share one 1024-wide copy.
        if single or qd % 2 == 0:
            ot = pmt.tile([128, 1024], BF16, tag="pt")
            octs[qd] = ot
            off = 0
        else:
            off = 512
        pt = octs[qd if (single or qd % 2 == 0) else qd - 1]
        ks = kscp.tile([128, 4, 128], BF16, tag="ks")
        for h in range(4):
            c, s = 2 * qd + h // 2, h % 2
            nc.vector.tensor_scalar(
                out=ks[:, h, :], in0=kin[:, c, s * 128:(s + 1) * 128],
                scalar1=rk[:, 2 * c + s:2 * c + s + 1], scalar2=None,
                op0=OP.mult)
            nc.tensor.transpose(
                out=pt[:, off + h * 128:off + (h + 1) * 128],
                in_=ks[:, h, :], identity=id128)
        done = single or qd % 2 == 1
        if done:
            src = pt[:, 0:512] if single else pt
            base = qd if single else qd - 1
            dst = kt[:, base * 512:(base + (1 if single else 2)) * 512]
            octs.pop(qd if single else qd - 1, None)
            if copy_eng == "act":
                nc.scalar.copy(out=dst, in_=src)
            else:
                nc.vector.tensor_copy(out=dst, in_=src)

    emit_knorms(0, 2)
    emit_rk(0, 2)
    emit_kprep_quad(0, "dve", single=True)
    emit_knorms(2, 4)
    emit_rk(2, 4)
    emit_kprep_quad(1, "dve", single=True)

    # ---- main loop, software-pipelined:
    #        exp(g) ; m(g+1) ; prep quad g+4 ; reduce(g)
    #      lse is split in two halves so the first half's tail overlaps ----
    lseA = plse.tile([OL, BIP], F32, tag="lse")
    lseB = plse.tile([OL, BIP], F32, tag="lse")

    def emit_tail(lse, h):
        # half h holds o = 8h..8h+7 in lse rows 0..7 (host remaps ecols)
        HO = OL // 2
        nc.scalar.activation(out=loglse, in_=lse[0:HO, :],
                             func=AF.Ln, bias=negnm[0:HO, h:h + 1], scale=1.0)
        nc.vector.tensor_reduce(
            out=sres[:, h * QB_CAP:(h + 1) * QB_CAP],
            in_=loglse.rearrange("p (b i) -> p b i", i=Lq),
            axis=mybir.AxisListType.X, op=OP.add)
        nc.sync.dma_start(out=out_d[h * HO:(h + 1) * HO, :],
                          in_=sres[:, h * QB_CAP:(h + 1) * QB_CAP])

    def emit_mains(g):
        T = pm.tile([128, 4 * BIP], F32, tag="mm")
        for s in range(4):
            ch = 4 * g + s
            nc.tensor.matmul(
                out=T[:, s * BIP:(s + 1) * BIP],
                lhsT=kt[:, ch * 128:(ch + 1) * 128],
                rhs=qt, start=True, stop=True)
        return T

    Ts = {0: emit_mains(0)}
    emit_knorms(4, 8)
    nc.vector.tensor_copy(out=negnm, in_=cdat[:, 384:386])  # bf16 -> f32

    for g in range(NG):
        e = etp.tile([128, 4 * BIP], BF16, tag="e")
        nc.scalar.activation(out=e, in_=Ts.pop(g), func=AF.Exp,
                             bias=0.0, scale=1.0)
        if g + 1 < NG:
            Ts[g + 1] = emit_mains(g + 1)
        lse = lseA if g < NG // 2 else lseB
        for s in range(4):
            o = (4 * g + s) // 2
            nc.tensor.matmul(
                out=lse[0:OL, :],
                lhsT=ecols[:, o * OL:(o + 1) * OL],
                rhs=e[:, s * BIP:(s + 1) * BIP],
                start=(g % (NG // 2) == 0 and s == 0),
                stop=(g % (NG // 2) == NG // 2 - 1 and s == 3))
        if g == 0:
            emit_rk(4, 8)
            emit_kprep_quad(2, "dve", single=True)
            emit_kprep_quad(3, "dve", single=True)
        elif g == 1:
            emit_knorms(8, 12)
            emit_rk(8, 12)
        elif g == 2:
            emit_kprep_quad(4, "dve", single=True)
            emit_kprep_quad(5, "dve", single=True)
        elif g == 3:
            emit_knorms(12, NKP)
            emit_rk(12, NKP)
            emit_kprep_quad(6, "dve", single=True)
        elif g == 4:
            emit_kprep_quad(7, "dve", single=True)
        if g == NG // 2 - 1:
            emit_tail(lseA, 0)
    emit_tail(lseB, 1)


def _patch_act_tables():
    """Make Bacc's act-table-load inserter pick one table serving both Exp
    and Ln (e.g. natural_log_exp_and_others) instead of thrashing between
    single-function tables: blank out any exp/ln table that doesn't contain
    both.  Entry positions (= act_func_set_id) are preserved."""
    import concourse.bacc as bacc_mod
    from concourse.hw_specs import get_activation_tables as gat
    if getattr(bacc_mod, "_act_tables_patched", False):
        return
    exp, ln = AF.Exp, AF.Ln

    def patched(arch):
        tabs = gat(arch)
        out = {}
        for name, s in tabs.items():
            has_e, has_l = exp in s, ln in s
            if (has_e or has_l) and not (has_e and has_l):
                s = s - {exp, ln}
            out[name] = s
        return out

    bacc_mod.get_activation_tables = patched
    bacc_mod._act_tables_patched = True


def build_program():
    _patch_act_tables()
    nc = bacc.Bacc("TRN2", target_bir_lowering=False, debug=False,
                   enable_asserts=False, num_devices=NCORES)
    q_d = nc.dram_tensor("q_in", [BIP, D], BF16, kind="ExternalInput").ap()
    k_d = nc.dram_tensor("k_in", [KR // 2, 2 * D], BF16, kind="ExternalInput").ap()
    cd_d = nc.dram_tensor("cdat", [128, 128 + OL * OL + 2], BF16,
                          kind="ExternalInput").ap()
    out_d = nc.dram_tensor("outp", [OL, QB_CAP], F32, kind="ExternalOutput").ap()

    with tile.TileContext(nc) as tc, ExitStack() as ctx:
        emit_kernel(ctx, tc, q_d, k_d, cd_d, out_d)
    nc.compile()
    return nc


def make_in_maps(q, k, q_mask, k_mask, OL_=None, ncores=NCORES):
    """Host-side shard prep.  Returns (passes, groups): passes is a list of
    per-core input-dict lists (one entry per device pass), groups the list of
    surviving batch indices handled by each pass."""
    q = np.asarray(q, dtype=np.float32)
    k = np.asarray(k, dtype=np.float32)
    q_mask = np.asarray(q_mask).astype(bool)
    k_mask = np.asarray(k_mask).astype(bool)

    surv = np.nonzero(~q_mask.any(axis=1))[0]
    groups = [surv[i:i + QB_CAP] for i in range(0, len(surv), QB_CAP)]
    if not groups:
        groups = [np.zeros((0,), dtype=np.int64)]

    kz = k.copy()
    kz[k_mask] = 0.0
    nmask = k_mask.sum(axis=1).astype(np.float32)           # [O]
    cdat0 = np.zeros((128, 128 + OL * OL + 2), dtype=BF16NP)
    cdat0[:, 0:128] = np.eye(128, dtype=BF16NP)
    for o in range(OL):
        # candidate o accumulates into lse row o % 8 of its half's tile
        cdat0[:, 128 + o * OL + (o % (OL // 2))] = 1.0

    core_static = []
    for c in range(ncores):
        osl = slice(c * OL, (c + 1) * OL)
        cdat = cdat0.copy()
        # negnm: exact small integers, representable in bf16; col h covers
        # the o's of half h
        nm = -nmask[osl]
        cdat[:OL // 2, -2] = nm[:OL // 2].astype(BF16NP)
        cdat[:OL // 2, -1] = nm[OL // 2:].astype(BF16NP)
        core_static.append({
            "k_in": np.ascontiguousarray(
                kz[osl].reshape(KR // 2, 2 * D).astype(BF16NP)),
            "cdat": cdat,
        })

    passes = []
    for g in groups:
        qp = np.zeros((BIP, D), dtype=np.float32)
        if len(g):
            qp[:len(g) * Lq] = q[g].reshape(len(g) * Lq, D)
        qp = qp.astype(BF16NP)
        in_maps = []
        for c in range(ncores):
            m = dict(core_static[c])
            m["q_in"] = qp
            in_maps.append(m)
        passes.append(in_maps)
    return passes, groups


def postprocess(per_pass_outs, groups, k_mask, logit_scale, ncores=NCORES):
    """Scatter per-pass [OL, QB_CAP] core results into the [B, O] output."""
    out = np.zeros((B, O), dtype=np.float32)
    coef = min(math.exp(float(logit_scale)), 100.0) / (
        ALPHA * (math.sqrt(Lq * Lk) + 1e-06))
    for outs, g in zip(per_pass_outs, groups):
        nb = len(g)
        if nb == 0:
            continue
        for c in range(ncores):
            blk = np.asarray(outs[c])[:, :nb].T * np.float32(coef)  # [nb, OL]
            out[np.asarray(g), c * OL:(c + 1) * OL] = blk
    out[:, np.asarray(k_mask).astype(bool).all(axis=1)] = 0.0
    return np.where(np.isfinite(out), out, 0.0).astype(np.float32)


_CACHED_NC = None


def kernel(q, k, q_mask, k_mask, logit_scale):
    global _CACHED_NC
    if _CACHED_NC is None:
        _CACHED_NC = build_program()
    passes, groups = make_in_maps(q, k, q_mask, k_mask)
    per_pass_outs = []
    for in_maps in passes:
        res = run_bass_kernel_spmd(_CACHED_NC, in_maps, list(range(NCORES)))
        per_pass_outs.append(
            [np.asarray(res.results[c]["outp"]) for c in range(NCORES)])
    return postprocess(per_pass_outs, groups, k_mask, logit_scale)


# revision 44
# speedup vs baseline: 10.0695x; 1.0396x over previous
"""ColBERT pairwise scoring kernel for 8x TRN2 NeuronCores.

Computation (see problem reference):
    qn = l2norm(q, axis=-1); kn = l2norm(k, axis=-1)
    S[b,o,i,j] = qn[b,i,:]·kn[o,j,:], masked positions -> -inf
    s[b,o] = sum_i logsumexp_j(ALPHA*S)/ALPHA, nonfinite -> 0
    out = s / (sqrt(Lq*Lk)+1e-6) * min(exp(logit_scale), 100)

Key observations exploited here:
  * Any batch row b with >= 1 masked query token is exactly 0 in the
    reference output (the -inf from that token survives the sum over Lq and
    is then zeroed).  Only rows with a fully-unmasked query need computing.
    The host packs those rows (up to QB_CAP=8 of them per pass) into a
    [256, D] tile; if more survive, the same program is run multiple times.
  * bf16 matmuls run 4x faster than fp32 on the PE (1 cycle/col vs 4), and
    |S| <= 1 with unit-norm rows, so bf16 inputs keep the overall relative
    error ~1e-3, far inside the 2e-2 gate.  No max-subtraction is needed for
    the logsumexp since |ALPHA*S| <= 12.

Sharding: candidate axis O split across 8 cores (OL=16 o's per core), packed
q replicated.  Per-core pipeline (all bf16 except PSUM):
    k arrives paired ([2048, 256] view of [4096, 128] so DMA descriptors are
    512B) -> row norms on GPSIMD -> rk = ALPHA/||k|| via exp(-0.5*ln) on ACT
    -> DVE scales k rows by rk -> PE transposes to kt[d, j] -> main matmul
    kt_chunk^T @ qt -> ACT exp over [128,1024] groups -> indicator-column
    matmul accumulates per-o sums into one PSUM tile -> Ln(sum - n_masked)
    -> sum over Lq -> DMA out [OL, QB_CAP].
Masked k rows are zeroed on the host (exp contributes exactly 1.0 there) and
the per-o masked count is subtracted inside the final Ln's bias.
"""

import math
import sys
from contextlib import ExitStack

import ml_dtypes
import numpy as np

BF16NP = ml_dtypes.bfloat16

for _p in ("/opt/trn_rl_repo",):
    if _p not in sys.path:
        sys.path.insert(0, _p)

import concourse.bass as bass
import concourse.bacc as bacc
import concourse.tile as tile
from concourse import mybir
from concourse.bass_utils import run_bass_kernel_spmd

ALPHA = 12.0
B, Lq, O, Lk, D = 64, 32, 128, 256, 128
NCORES = 8
OL = O // NCORES          # candidates per core = 16
KR = OL * Lk              # k rows per core = 4096
NKP = KR // 256           # paired k chunks per core = 16
QB_CAP = 8                # max surviving batch rows per pass
BIP = QB_CAP * Lq         # packed query rows = 256

F32 = mybir.dt.float32
BF16 = mybir.dt.bfloat16
AF = mybir.ActivationFunctionType
OP = mybir.AluOpType


def emit_kernel(ctx, tc, q_d, k_d, cd_d, out_d):
    nc = tc.nc
    NCH = KR // 128           # 128-row j chunks = 32
    NG = NCH // 4             # exp groups of 4 chunks = 8
    NQD = 4                   # k arrives in 4 quarter DMAs
    QP = NKP // NQD           # paired chunks per quarter = 4

    sing = ctx.enter_context(tc.tile_pool(name="sing", bufs=1))
    kscp = ctx.enter_context(tc.tile_pool(name="ksc", bufs=4))
    sqp = ctx.enter_context(tc.tile_pool(name="sq", bufs=6))
    pmt = ctx.enter_context(tc.tile_pool(name="pmt", bufs=2, space="PSUM"))
    pm = ctx.enter_context(tc.tile_pool(name="pm", bufs=2, space="PSUM"))
    plse = ctx.enter_context(tc.tile_pool(name="plse", bufs=2, space="PSUM"))
    etp = ctx.enter_context(tc.tile_pool(name="et", bufs=2))

    kin = sing.tile([128, NKP, 256], BF16)   # paired natural k
    qin = sing.tile([128, 2, 128], BF16)     # natural packed q
    kt = sing.tile([128, KR], BF16)          # scaled transposed k [d, j]
    qt = sing.tile([128, BIP], BF16)         # normalized transposed q [d, bi]
    nk = sing.tile([128, 2 * NKP], F32)      # k row squared norms
    rk = sing.tile([128, 2 * NKP], F32)      # ALPHA / ||k row||
    nq = sing.tile([128, 2], F32)
    rq = sing.tile([128, 2], F32)
    cdat = sing.tile([128, 128 + OL * OL + 2], BF16)  # id128 ++ ecols ++ negnm
    id128 = cdat[:, 0:128]
    ecols = cdat[:, 128:128 + OL * OL]
    negnm = sing.tile([128, 2], F32)   # col h: -n_masked for o in half h
    loglse = sing.tile([OL // 2, BIP], F32)
    sres = sing.tile([OL // 2, 2 * QB_CAP], F32)
    bias_eps = sing.tile([128, 1], F32)      # 1e-30, keeps Ln input nonzero
    bias_lna = sing.tile([128, 1], F32)      # ln(ALPHA), folds alpha into rk
    nc.vector.memset(bias_eps, 1e-30)
    nc.vector.memset(bias_lna, math.log(ALPHA))

    # ---- inputs in: q first (its prep finishes before k lands), then k
    #      quarters, all on SP; constants ride the ACT queue ----
    kre = k_d.rearrange("(c p) e -> p c e", p=128)
    nc.sync.dma_start(out=qin, in_=q_d.rearrange("(c p) d -> p c d", p=128))
    for qd in range(NQD):
        nc.sync.dma_start(out=kin[:, qd * QP:(qd + 1) * QP, :],
                          in_=kre[:, qd * QP:(qd + 1) * QP, :])
    nc.scalar.dma_start(out=cdat, in_=cd_d)

    # ---- helpers ----
    def emit_knorms(p0, p1):
        for c in range(p0, p1):
            for s in range(2):
                sq = sqp.tile([128, 128], BF16, tag="sq")
                nc.vector.scalar_tensor_tensor(
                    out=sq, in0=kin[:, c, s * 128:(s + 1) * 128], scalar=1.0,
                    in1=kin[:, c, s * 128:(s + 1) * 128],
                    op0=OP.mult, op1=OP.mult,
                    accum_out=nk[:, 2 * c + s:2 * c + s + 1])

    def emit_rk(c0, c1):
        # rk = ALPHA * exp(-0.5 * ln(n2)): stays within the ln/exp ACT table
        nc.scalar.activation(out=nk[:, 2 * c0:2 * c1], in_=nk[:, 2 * c0:2 * c1],
                             func=AF.Ln, bias=bias_eps[:, 0:1], scale=1.0)
        nc.scalar.activation(out=rk[:, 2 * c0:2 * c1], in_=nk[:, 2 * c0:2 * c1],
                             func=AF.Exp, bias=bias_lna[:, 0:1], scale=-0.5)

    # ---- q prep first (qin lands before k), then per-quad norm->rk->prep
    #      so quad 0 feeds the first main matmul as early as possible ----
    for c in range(2):
        sq = sqp.tile([128, 128], BF16, tag="sq")
        nc.vector.scalar_tensor_tensor(
            out=sq, in0=qin[:, c, :], scalar=1.0, in1=qin[:, c, :],
            op0=OP.mult, op1=OP.mult, accum_out=nq[:, c:c + 1])
    nc.scalar.activation(out=nq, in_=nq, func=AF.Ln,
                         bias=bias_eps[:, 0:1], scale=1.0)
    nc.scalar.activation(out=rq, in_=nq, func=AF.Exp, bias=0.0, scale=-0.5)
    qs = kscp.tile([128, 2, 128], BF16, tag="qs")
    pq = pmt.tile([128, 1024], BF16, tag="pt")
    for c in range(2):
        nc.vector.tensor_scalar(out=qs[:, c, :], in0=qin[:, c, :],
                                scalar1=rq[:, c:c + 1], scalar2=None,
                                op0=OP.mult)
        nc.tensor.transpose(out=pq[:, c * 128:(c + 1) * 128],
                            in_=qs[:, c, :], identity=id128)
    nc.vector.tensor_copy(out=qt, in_=pq[:, 0:256])

    # k prep: per quad qd (pairs 2qd, 2qd+1 -> kt cols qd*512..) norms, rk,
    # scale, transpose into an "oct" [128, 1024] PSUM tile shared by quad
    # pairs; one copy per oct (ACT pre-loop where it idles, DVE mid-loop).
    octs = {}

    def emit_kprep_quad(qd, copy_eng, single=False):
        # single=True: quad gets its own [128, 512] copy right away (lower
        # latency for the first mains); else pairs share one 1024-wide copy.
        if single or qd % 2 == 0:
            ot = pmt.tile([128, 1024], BF16, tag="pt")
            octs[qd] = ot
            off = 0
        else:
            off = 512
        pt = octs[qd if (single or qd % 2 == 0) else qd - 1]
        ks = kscp.tile([128, 4, 128], BF16, tag="ks")
        for h in range(4):
            c, s = 2 * qd + h // 2, h % 2
            nc.vector.tensor_scalar(
                out=ks[:, h, :], in0=kin[:, c, s * 128:(s + 1) * 128],
                scalar1=rk[:, 2 * c + s:2 * c + s + 1], scalar2=None,
                op0=OP.mult)
            nc.tensor.transpose(
                out=pt[:, off + h * 128:off + (h + 1) * 128],
                in_=ks[:, h, :], identity=id128)
        done = single or qd % 2 == 1
        if done:
            src = pt[:, 0:512] if single else pt
            base = qd if single else qd - 1
            dst = kt[:, base * 512:(base + (1 if single else 2)) * 512]
            octs.pop(qd if single else qd - 1, None)
            if copy_eng == "act":
                nc.scalar.copy(out=dst, in_=src)
            else:
                nc.vector.tensor_copy(out=dst, in_=src)

    emit_knorms(0, 2)
    emit_rk(0, 2)
    emit_kprep_quad(0, "act", single=True)
    emit_knorms(2, 4)
    emit_rk(2, 4)
    emit_kprep_quad(1, "act", single=True)

    # ---- main loop, software-pipelined:
    #        exp(g) ; m(g+1) ; prep quad g+4 ; reduce(g)
    #      lse is split in two halves so the first half's tail overlaps ----
    lseA = plse.tile([OL, BIP], F32, tag="lse")
    lseB = plse.tile([OL, BIP], F32, tag="lse")

    def emit_tail(lse, h):
        # half h holds o = 8h..8h+7 in lse rows 0..7 (host remaps ecols)
        HO = OL // 2
        nc.scalar.activation(out=loglse, in_=lse[0:HO, :],
                             func=AF.Ln, bias=negnm[0:HO, h:h + 1], scale=1.0)
        nc.vector.tensor_reduce(
            out=sres[:, h * QB_CAP:(h + 1) * QB_CAP],
            in_=loglse.rearrange("p (b i) -> p b i", i=Lq),
            axis=mybir.AxisListType.X, op=OP.add)
        nc.sync.dma_start(out=out_d[h * HO:(h + 1) * HO, :],
                          in_=sres[:, h * QB_CAP:(h + 1) * QB_CAP])

    def emit_mains(g):
        T = pm.tile([128, 4 * BIP], F32, tag="mm")
        for s in range(4):
            ch = 4 * g + s
            nc.tensor.matmul(
                out=T[:, s * BIP:(s + 1) * BIP],
                lhsT=kt[:, ch * 128:(ch + 1) * 128],
                rhs=qt, start=True, stop=True)
        return T

    Ts = {0: emit_mains(0)}
    emit_knorms(4, 8)
    nc.vector.tensor_copy(out=negnm, in_=cdat[:, 384:386])  # bf16 -> f32

    for g in range(NG):
        e = etp.tile([128, 4 * BIP], BF16, tag="e")
        nc.scalar.activation(out=e, in_=Ts.pop(g), func=AF.Exp,
                             bias=0.0, scale=1.0)
        if g + 1 < NG:
            Ts[g + 1] = emit_mains(g + 1)
        lse = lseA if g < NG // 2 else lseB
        for s in range(4):
            o = (4 * g + s) // 2
            nc.tensor.matmul(
                out=lse[0:OL, :],
                lhsT=ecols[:, o * OL:(o + 1) * OL],
                rhs=e[:, s * BIP:(s + 1) * BIP],
                start=(g % (NG // 2) == 0 and s == 0),
                stop=(g % (NG // 2) == NG // 2 - 1 and s == 3))
        if g == 0:
            emit_rk(4, 8)
            emit_kprep_quad(2, "dve", single=True)
            emit_kprep_quad(3, "dve", single=True)
        elif g == 1:
            emit_knorms(8, 12)
            emit_rk(8, 12)
        elif g == 2:
            emit_kprep_quad(4, "dve", single=True)
            emit_kprep_quad(5, "dve", single=True)
        elif g == 3:
            emit_knorms(12, NKP)
            emit_rk(12, NKP)
            emit_kprep_quad(6, "dve", single=True)
        elif g == 4:
            emit_kprep_quad(7, "dve", single=True)
        if g == NG // 2 - 1:
            emit_tail(lseA, 0)
    emit_tail(lseB, 1)


def _patch_act_tables():
    """Make Bacc's act-table-load inserter pick one table serving both Exp
    and Ln (e.g. natural_log_exp_and_others) instead of thrashing between
    single-function tables: blank out any exp/ln table that doesn't contain
    both.  Entry positions (= act_func_set_id) are preserved."""
    import concourse.bacc as bacc_mod
    from concourse.hw_specs import get_activation_tables as gat
    if getattr(bacc_mod, "_act_tables_patched", False):
        return
    exp, ln = AF.Exp, AF.Ln

    def patched(arch):
        tabs = gat(arch)
        out = {}
        for name, s in tabs.items():
            has_e, has_l = exp in s, ln in s
            if (has_e or has_l) and not (has_e and has_l):
                s = s - {exp, ln}
            out[name] = s
        return out

    bacc_mod.get_activation_tables = patched
    bacc_mod._act_tables_patched = True


def build_program():
    _patch_act_tables()
    nc = bacc.Bacc("TRN2", target_bir_lowering=False, debug=False,
                   enable_asserts=False, num_devices=NCORES)
    q_d = nc.dram_tensor("q_in", [BIP, D], BF16, kind="ExternalInput").ap()
    k_d = nc.dram_tensor("k_in", [KR // 2, 2 * D], BF16, kind="ExternalInput").ap()
    cd_d = nc.dram_tensor("cdat", [128, 128 + OL * OL + 2], BF16,
                          kind="ExternalInput").ap()
    out_d = nc.dram_tensor("outp", [OL, QB_CAP], F32, kind="ExternalOutput").ap()

    with tile.TileContext(nc) as tc, ExitStack() as ctx:
        emit_kernel(ctx, tc, q_d, k_d, cd_d, out_d)
    nc.compile()
    return nc


def make_in_maps(q, k, q_mask, k_mask, OL_=None, ncores=NCORES):
    """Host-side shard prep.  Returns (passes, groups): passes is a list of
    per-core input-dict lists (one entry per device pass), groups the list of
    surviving batch indices handled by each pass."""
    q = np.asarray(q, dtype=np.float32)
    k = np.asarray(k, dtype=np.float32)
    q_mask = np.asarray(q_mask).astype(bool)
    k_mask = np.asarray(k_mask).astype(bool)

    surv = np.nonzero(~q_mask.any(axis=1))[0]
    groups = [surv[i:i + QB_CAP] for i in range(0, len(surv), QB_CAP)]
    if not groups:
        groups = [np.zeros((0,), dtype=np.int64)]

    kz = k.copy()
    kz[k_mask] = 0.0
    nmask = k_mask.sum(axis=1).astype(np.float32)           # [O]
    cdat0 = np.zeros((128, 128 + OL * OL + 2), dtype=BF16NP)
    cdat0[:, 0:128] = np.eye(128, dtype=BF16NP)
    for o in range(OL):
        # candidate o accumulates into lse row o % 8 of its half's tile
        cdat0[:, 128 + o * OL + (o % (OL // 2))] = 1.0

    core_static = []
    for c in range(ncores):
        osl = slice(c * OL, (c + 1) * OL)
        cdat = cdat0.copy()
        # negnm: exact small integers, representable in bf16; col h covers
        # the o's of half h
        nm = -nmask[osl]
        cdat[:OL // 2, -2] = nm[:OL // 2].astype(BF16NP)
        cdat[:OL // 2, -1] = nm[OL // 2:].astype(BF16NP)
        core_static.append({
            "k_in": np.ascontiguousarray(
                kz[osl].reshape(KR // 2, 2 * D).astype(BF16NP)),
            "cdat": cdat,
        })

    passes = []
    for g in groups:
        qp = np.zeros((BIP, D), dtype=np.float32)
        if len(g):
            qp[:len(g) * Lq] = q[g].reshape(len(g) * Lq, D)
        qp = qp.astype(BF16NP)
        in_maps = []
        for c in range(ncores):
            m = dict(core_static[c])
            m["q_in"] = qp
            in_maps.append(m)
        passes.append(in_maps)
    return passes, groups


def postprocess(per_pass_outs, groups, k_mask, logit_scale, ncores=NCORES):
    """Scatter per-pass [OL, QB_CAP] core results into the [B, O] output."""
    out = np.zeros((B, O), dtype=np.float32)
    coef = min(math.exp(float(logit_scale)), 100.0) / (
        ALPHA * (math.sqrt(Lq * Lk) + 1e-06))
    for outs, g in zip(per_pass_outs, groups):
        nb = len(g)
        if nb == 0:
            continue
        for c in range(ncores):
            blk = np.asarray(outs[c])[:, :nb].T * np.float32(coef)  # [nb, OL]
            out[np.asarray(g), c * OL:(c + 1) * OL] = blk
    out[:, np.asarray(k_mask).astype(bool).all(axis=1)] = 0.0
    return np.where(np.isfinite(out), out, 0.0).astype(np.float32)


_CACHED_NC = None


def kernel(q, k, q_mask, k_mask, logit_scale):
    global _CACHED_NC
    if _CACHED_NC is None:
        _CACHED_NC = build_program()
    passes, groups = make_in_maps(q, k, q_mask, k_mask)
    per_pass_outs = []
    for in_maps in passes:
        res = run_bass_kernel_spmd(_CACHED_NC, in_maps, list(range(NCORES)))
        per_pass_outs.append(
            [np.asarray(res.results[c]["outp"]) for c in range(NCORES)])
    return postprocess(per_pass_outs, groups, k_mask, logit_scale)
